# revision 16
# baseline (speedup 1.0000x reference)
"""Trainium2 Bass kernel for the CNN/segment-reduce model.

Strategy (pure data-parallel over batch, 8 cores x 64 batch elems):
  host:   the conv over the 100 pos-embedding channels is an exact table
          lookup (conv_w @ emb gathered by index), computed on host along
          with the 4 leftover W channels (ci 896..999) -> ye term; the
          device contraction shrinks from 8 to 7 ci-chunks (-12.5% PE).
          Remaining host prep: transpose/zero-pad conv input [896, 134],
          segment weight-masks, fc_w column reorder (fc_b folded via a
          constant-1 feature), PE-facing data in bf16.
  device: conv = PE matmuls, contraction over 7 ci-chunks x k taps with
          the [128ci,128co] weight block stationary; rhs = shifted x
          window over 4 batch elems (N=508); accumulate in PSUM.
          DVE adds the host ye tile to PSUM (th_in f32), ACT tanh(+bias)
          -> bf16; DVE mask-multiply + segment-reduce -> feature tile;
          FC = 23 accumulating matmuls into PSUM [64b, 19], issued
          fine-grained per conv group for the last batch half to shrink
          the PE tail; fused exp/sum softmax; DMA out fp32.
"""

import numpy as np
import ml_dtypes

B, S, DW, DP, DC, VP, VR = 512, 128, 300, 50, 256, 256, 19
KS = (3, 5, 7)
CIN = 3 * DW + 2 * DP  # 1000
CDEV = 896             # ci channels contracted on device (7 chunks)
NCH = CDEV // 128      # 7 contraction chunks
NCORE = 8
BC = B // NCORE        # 64 batch elems per core
NB = 16                # batch elems per resident x group
NBG = BC // NB         # 4 groups
NQ = NB // 4           # 4 psum quads (4 b per N=508 matmul)
LW = S + 6             # 3 zero cols each side
PAD = 3
SL = S - 1             # conv cols actually consumed (l=127 never pooled)
NF = 4 * SL            # matmul free size (4 batch elems)
NSETS = sum(k * NCH for k in KS) * 2          # 210 weight blocks
NCHK = 23                                     # feature chunks of 128
F_CONV = 18 * 128                             # 2304 conv features
BF16 = ml_dtypes.bfloat16

# weight block index: ordered (ki-major, h, t, c)
_PREFIX = {}
_off = 0
for _ki, _k in enumerate(KS):
    for _h in range(2):
        _PREFIX[(_ki, _h)] = _off
        _off += _k * NCH


def _bidx(ki, h, t, c):
    # c-major within a group: matches the device's set consumption order so
    # a partial weight DMA unblocks the first matmuls immediately
    return _PREFIX[(ki, h)] + c * KS[ki] + t


def _build_program():
    from contextlib import ExitStack
    import concourse.tile as tile
    from concourse import bacc, mybir

    f32 = mybir.dt.float32
    bf16 = mybir.dt.bfloat16
    AF = mybir.ActivationFunctionType
    ALU = mybir.AluOpType
    AX = mybir.AxisListType

    nc = bacc.Bacc("TRN2", target_bir_lowering=False, debug=False,
                   num_devices=NCORE)

    X = nc.declare_dram_parameter("X", [NBG, NCH, 128, NB * LW], bf16,
                                  isOutput=False)
    WM = nc.declare_dram_parameter("WM", [NBG, 128, 3 * NB * SL], bf16,
                                   isOutput=False)
    YE = nc.declare_dram_parameter("YE", [NBG, 6, 128, NB * SL], bf16,
                                   isOutput=False)
    WT = nc.declare_dram_parameter("WT", [128, NSETS * 128], bf16,
                                   isOutput=False)
    FCW = nc.declare_dram_parameter("FCW", [128, NCHK * VR], bf16,
                                    isOutput=False)
    E12 = nc.declare_dram_parameter("E12", [5 * 128, BC], f32, isOutput=False)
    CB = nc.declare_dram_parameter("CB", [128, 6], f32, isOutput=False)
    OUT = nc.declare_dram_parameter("OUT", [BC, VR], f32, isOutput=True)

    with tile.TileContext(nc) as tc, ExitStack() as ctx:
        const = ctx.enter_context(tc.tile_pool(name="const", bufs=1))
        xpool = ctx.enter_context(tc.tile_pool(name="xp", bufs=2))
        mpool = ctx.enter_context(tc.tile_pool(name="mp", bufs=2))
        yepool = ctx.enter_context(tc.tile_pool(name="yep", bufs=4))
        tpool = ctx.enter_context(tc.tile_pool(name="tp", bufs=4))
        thpool = ctx.enter_context(tc.tile_pool(name="thp", bufs=4))
        prpool = ctx.enter_context(tc.tile_pool(name="prp", bufs=4))
        pspool = ctx.enter_context(tc.tile_pool(name="psp", bufs=8,
                                                space="PSUM"))

        # PE warm-up: dummy matmuls with no DMA deps so the HAM clock-gate
        # flips to 8/8 while the first weight/x DMAs are in flight.
        warm = const.tile([128, NF], bf16)
        nc.any.memset(warm[:], 0.0)
        wps = pspool.tile([128, NF], f32, tag="ps")
        for _ in range(14):
            nc.tensor.matmul(wps[:], warm[:, 0:128], warm[:],
                             start=True, stop=True)

        wt_sb = const.tile([128, NSETS * 128], bf16)
        wt_bounds = sorted(_PREFIX.values()) + [NSETS]
        fcw_sb = const.tile([128, NCHK * VR], bf16)
        cb_sb = const.tile([128, 6], f32)
        feat32 = const.tile([128, NCHK * BC], f32)
        featbf = const.tile([128, NCHK * BC], bf16)

        fcps = pspool.tile([BC, VR], f32, tag="ps")
        mx = const.tile([BC, 1], f32)
        esm = const.tile([BC, VR], f32)
        ssum = const.tile([BC, 1], f32)
        rin = const.tile([BC, 1], f32)
        osb = const.tile([BC, VR], f32)

        def softmax_rows(b0, b1):
            # softmax + output DMA for batch rows [b0, b1); row half 0 runs
            # mid-kernel as soon as its FC accumulation closes.  No max
            # subtraction: |logits| < ~5 here, exp is safe in f32, and
            # skipping it drops a reduce + the ACT accumulator drain from
            # the end-of-kernel critical chain.
            nc.scalar.activation(esm[b0:b1, :], fcps[b0:b1, :], AF.Exp)
            nc.vector.tensor_reduce(ssum[b0:b1], esm[b0:b1, :], axis=AX.X,
                                    op=ALU.add)
            nc.vector.reciprocal(rin[b0:b1], ssum[b0:b1])
            nc.vector.tensor_scalar_mul(osb[b0:b1, :], esm[b0:b1, :],
                                        rin[b0:b1])
            nc.sync.dma_start(OUT.ap()[b0:b1, :], osb[b0:b1, :])

        for bg in range(NBG):
            x_sb = xpool.tile([128, NCH * NB * LW], bf16, tag="x",
                              name=f"x_sb_{bg}")
            if bg == 0:
                # x chunk 0 (first quad first) + the first few weight
                # blocks race in; blocks are in consumption order, so 6
                # blocks cover the first two x chunks' worth of matmuls.
                # GpSimd issues the later x chunks in parallel with Sync's
                # weight stream — one engine alone serializes ~0.6us per
                # DMA descriptor and the cold-start conv eats the delay.
                nc.sync.dma_start(x_sb[:, 0:4 * LW], X.ap()[0, 0][:, 0:4 * LW])
                nc.sync.dma_start(wt_sb[:, 0:3 * 128], WT.ap()[:, 0:3 * 128])
                nc.sync.dma_start(wt_sb[:, 3 * 128:6 * 128],
                                  WT.ap()[:, 3 * 128:6 * 128])
                nc.sync.dma_start(x_sb[:, 4 * LW:NB * LW],
                                  X.ap()[0, 0][:, 4 * LW:NB * LW])
                # x chunk 1 ahead of the bulk weight blocks: the c-major
                # matmul order consumes x chunks faster than wt groups
                nc.sync.dma_start(x_sb[:, NB * LW:2 * NB * LW],
                                  X.ap()[0, 1])
                nc.sync.dma_start(wt_sb[:, 6 * 128:wt_bounds[1] * 128],
                                  WT.ap()[:, 6 * 128:wt_bounds[1] * 128])
            xq = nc.gpsimd if bg == 0 else nc.sync
            for c in range(0 if bg else 2, NCH):
                xq.dma_start(x_sb[:, c * NB * LW:(c + 1) * NB * LW],
                             X.ap()[bg, c])
            wm_sb = mpool.tile([128, 3 * NB * SL], bf16, tag="wm",
                               name=f"wm_sb_{bg}")
            xq.dma_start(wm_sb[:], WM.ap()[bg])
            ye_g0 = yepool.tile([128, NB * SL], bf16, tag="ye",
                                name=f"ye_{bg}_0")
            xq.dma_start(ye_g0[:], YE.ap()[bg, 0])
            if bg == 0:
                nc.gpsimd.dma_start(cb_sb[:], CB.ap()[:])
                # stream the rest of the weights in behind x/wm for bg0
                for wi in range(1, len(wt_bounds) - 1):
                    nc.sync.dma_start(
                        wt_sb[:, wt_bounds[wi] * 128:wt_bounds[wi + 1] * 128],
                        WT.ap()[:, wt_bounds[wi] * 128:wt_bounds[wi + 1] * 128])
                nc.sync.dma_start(fcw_sb[:], FCW.ap()[:])
                for j in range(5):
                    nc.gpsimd.dma_start(
                        feat32[:, (18 + j) * BC:(19 + j) * BC],
                        E12.ap()[j * 128:(j + 1) * 128, :])
            xv = x_sb[:].rearrange("p (c b w) -> p c b w", c=NCH, b=NB)

            def pool_span(bg, ki, h, b0, nb, ps, ye_sb):
                # mask-pool nb batch elems starting at b0 (within this bg)
                g = ki * 2 + h
                nf = nb * SL
                th_in = thpool.tile([128, nf], f32, tag="ti",
                                    name=f"ti_{bg}_{ki}_{h}_{b0}")
                nc.vector.tensor_tensor(th_in[:], ps[:],
                                        ye_sb[:, b0 * SL:(b0 + nb) * SL],
                                        op=ALU.add)
                th = tpool.tile([128, nf], bf16, tag="th",
                                name=f"th_{bg}_{ki}_{h}_{b0}")
                nc.scalar.activation(th[:], th_in[:], AF.Tanh,
                                     bias=cb_sb[:, g:g + 1])
                for seg in range(3):
                    ch = ki * 6 + seg * 2 + h
                    col0 = ch * BC + bg * NB + b0
                    wmq = wm_sb[:, seg * NB * SL + b0 * SL:
                                seg * NB * SL + (b0 + nb) * SL]
                    pr = prpool.tile([128, nf], bf16, tag="pr",
                                     name=f"pr_{bg}_{ki}_{h}_{b0}_{seg}")
                    nc.vector.tensor_tensor(pr[:], th[:], wmq, op=ALU.mult)
                    nc.vector.tensor_reduce(
                        feat32[:, col0:col0 + nb],
                        pr[:].rearrange("p (b w) -> p b w", w=SL),
                        axis=AX.X, op=ALU.add)

            def conv_group(bg, ki, h, ye_sb, fine_tail=False):
                k = KS[ki]
                nsets = k * NCH
                # c-major: early sets touch only early x chunks, so the
                # cold-start conv isn't gated on the full x DMA
                sets = [(t, c) for c in range(NCH) for t in range(k)]

                def mm(ps, b0, nb, si, t, c):
                    s = t - k // 2
                    bi = _bidx(ki, h, t, c)
                    rhs = xv[:, c, b0:b0 + nb, PAD + s:PAD + s + SL]
                    nc.tensor.matmul(ps[:], wt_sb[:, bi * 128:(bi + 1) * 128],
                                     rhs, start=(si == 0),
                                     stop=(si == nsets - 1))

                if fine_tail:
                    # final group: span-outer staggered with shrinking span
                    # widths, so the pooling chain exposed after the very
                    # last matmul is as short as possible
                    spans = [(0, 4), (4, 4), (8, 4), (12, 2), (14, 1),
                             (15, 1)]
                    for b0, nb in spans:
                        ps = pspool.tile([128, nb * SL], f32, tag="ps",
                                         name=f"psd_{bg}_{ki}_{h}_{b0}")
                        for si, (t, c) in enumerate(sets):
                            mm(ps, b0, nb, si, t, c)
                        pool_span(bg, ki, h, b0, nb, ps, ye_sb)
                else:
                    pss = [pspool.tile([128, NF], f32, tag="ps",
                                       name=f"ps_{bg}_{ki}_{h}_{q}")
                           for q in range(NQ)]
                    for si, (t, c) in enumerate(sets):
                        for q in range(NQ):
                            mm(pss[q], q * 4, 4, si, t, c)
                    for q in range(NQ):
                        pool_span(bg, ki, h, q * 4, 4, pss[q], ye_sb)

            def cast_cols(b0, nb):
                nc.vector.tensor_copy(
                    featbf[:].rearrange("p (c b) -> p c b",
                                        c=NCHK)[:, :, b0:b0 + nb],
                    feat32[:].rearrange("p (c b) -> p c b",
                                        c=NCHK)[:, :, b0:b0 + nb])

            def cast_grp(ki, h, b0, nb):
                # cast only this conv group's 3 feature chunks for b0..b0+nb
                ch0 = ki * 6 + h
                nc.vector.tensor_copy(
                    featbf[:].rearrange("p (c b) -> p c b",
                                        c=NCHK)[:, ch0:ch0 + 5:2, b0:b0 + nb],
                    feat32[:].rearrange("p (c b) -> p c b",
                                        c=NCHK)[:, ch0:ch0 + 5:2, b0:b0 + nb])

            def cast_e12(b0, nb):
                nc.vector.tensor_copy(
                    featbf[:].rearrange("p (c b) -> p c b",
                                        c=NCHK)[:, 18:, b0:b0 + nb],
                    feat32[:].rearrange("p (c b) -> p c b",
                                        c=NCHK)[:, 18:, b0:b0 + nb])

            def fc_mm(rows, ch, start, stop):
                b0, b1 = rows
                nc.tensor.matmul(
                    fcps[b0:b1, :],
                    featbf[:, ch * BC + b0:ch * BC + b1],
                    fcw_sb[:, ch * VR:(ch + 1) * VR],
                    start=start, stop=stop)

            def fc_half(half):
                b0 = half * 32
                for ch in range(NCHK):
                    fc_mm((b0, b0 + 32), ch, ch == 0, ch == NCHK - 1)

            for gi, (ki, h) in enumerate([(ki, h) for ki in range(3)
                                          for h in range(2)]):
                final = (bg == NBG - 1) and (ki, h) == (2, 1)
                if gi > 0:
                    ye_sb = yepool.tile([128, NB * SL], bf16, tag="ye",
                                        name=f"ye_{bg}_{gi}")
                    nc.sync.dma_start(ye_sb[:], YE.ap()[bg, gi])
                else:
                    ye_sb = ye_g0
                conv_group(bg, ki, h, ye_sb, fine_tail=final)
                if bg == 2 and gi == 0:
                    # batch half 0 fully pooled at end of bg1; cast dep is
                    # ready by now so no PE bubble
                    cast_cols(0, 32)
                    fc_half(0)
                if bg == 2 and gi == 1:
                    softmax_rows(0, 32)
                if bg == 3:
                    # fine-grained FC for b32..63: e1/e2 chunks up front,
                    # then each conv group's 3 chunks as soon as the group
                    # is pooled -> tiny PE tail after the last conv matmul
                    if gi == 0:
                        cast_cols(32, 16)  # b 32..47 pooled at end of bg2
                        cast_e12(48, 16)
                        for j, ch in enumerate(range(18, 23)):
                            fc_mm((32, 64), ch, j == 0, False)
                    cast_grp(ki, h, 48, 16)
                    for j, seg in enumerate(range(3)):
                        ch = ki * 6 + seg * 2 + h
                        fc_mm((32, 64), ch, False,
                              gi == 5 and j == 2)

        softmax_rows(32, BC)

    nc.compile()
    return nc


_NC_CACHE = []


def _get_program():
    if not _NC_CACHE:
        _NC_CACHE.append(_build_program())
    return _NC_CACHE[0]


def _host_ye(W, pos_emb1, pos_emb2, conv_ws, W_pos1, W_pos2):
    """Exact host-side conv contribution of ci 896..999: the 100
    pos-embedding channels collapse to per-tap table lookups
    (conv_w @ emb.T gathered by index), plus a 4-channel einsum."""
    idx1 = W_pos1.astype(np.int64)
    idx2 = W_pos2.astype(np.int64)
    e1m = pos_emb1.astype(np.float32)
    e2m = pos_emb2.astype(np.float32)
    W4 = W[:, :, 896:900].astype(np.float32)       # [B, S, 4]
    ye = np.empty((B, 6, 128, SL), np.float32)
    for ki, k in enumerate(KS):
        cw = conv_ws[ki].astype(np.float32)        # [DC, CIN, k]
        y = np.zeros((B, SL, DC), np.float32)
        for t in range(k):
            s = t - k // 2
            l0, l1 = max(0, -s), min(SL, S - s)
            T1 = np.ascontiguousarray((cw[:, 900:950, t] @ e1m.T).T)
            T2 = np.ascontiguousarray((cw[:, 950:1000, t] @ e2m.T).T)
            y[:, l0:l1] += T1[idx1[:, l0 + s:l1 + s]]
            y[:, l0:l1] += T2[idx2[:, l0 + s:l1 + s]]
            y[:, l0:l1] += W4[:, l0 + s:l1 + s] @ cw[:, 896:900, t].T
        for h in range(2):
            ye[:, ki * 2 + h] = y[:, :, h * 128:(h + 1) * 128].transpose(
                0, 2, 1)
    return ye


def _prep_inputs(W, e1, e2, pos_emb1, pos_emb2, conv_ws, conv_bs, fc_w, fc_b,
                 W_pos1, W_pos2, e1_p, e2_p):
    """Host-side data layout; returns per-core input maps."""
    # --- conv input: [B, 896, 134] zero-padded, 3 zero cols each side ---
    Xf = W[:, :, :CDEV].transpose(0, 2, 1)         # [B, 896, S]
    Xpad = np.zeros((B, CDEV, LW), np.float32)
    Xpad[:, :, PAD:PAD + S] = Xf
    Xpad = Xpad.astype(BF16).reshape(NCORE, NBG, NB, NCH, 128, LW)
    Xc = np.ascontiguousarray(Xpad.transpose(0, 1, 3, 4, 2, 5)).reshape(
        NCORE, NBG, NCH, 128, NB * LW)

    # --- host ye: pos-emb table-lookup conv + leftover 4 channels ---
    ye = _host_ye(W, pos_emb1, pos_emb2, conv_ws, W_pos1, W_pos2)
    ye = ye.astype(BF16).reshape(NCORE, NBG, NB, 6, 128, SL)
    YEc = np.ascontiguousarray(ye.transpose(0, 1, 3, 4, 2, 5)).reshape(
        NCORE, NBG, 6, 128, NB * SL)

    # --- segment weight masks m/cnt, replicated over 128 partitions ---
    d1 = np.minimum(e1_p, e2_p).astype(np.int64)
    d2 = np.maximum(e1_p, e2_p).astype(np.int64)
    idx = np.arange(S)[None, :]
    m1 = (idx < d1[:, None])
    m2 = (idx >= d1[:, None]) & (idx < d2[:, None])
    m3 = (idx >= d2[:, None]) & (idx < S - 1)
    wm = np.stack([m1, m2, m3], axis=1).astype(np.float32)  # [B,3,S]
    cnt = np.maximum(wm.sum(axis=2), 1.0)
    wm /= cnt[:, :, None]
    wm = wm[:, :, :SL]  # l=127 is never pooled
    wm = wm.astype(BF16).reshape(NCORE, NBG, NB, 3, SL)
    wm = np.ascontiguousarray(wm.transpose(0, 1, 3, 2, 4)).reshape(
        NCORE, NBG, 1, 3 * NB * SL)
    WMc = np.ascontiguousarray(np.broadcast_to(
        wm, (NCORE, NBG, 128, 3 * NB * SL)))

    # --- conv weights -> stationary blocks [128ci, 128co], bf16 ---
    wt = np.zeros((128, NSETS * 128), np.float32)
    for ki, k in enumerate(KS):
        cw = conv_ws[ki][:, :CDEV, :]
        for h in range(2):
            for t in range(k):
                for c in range(NCH):
                    blk = cw[h * 128:(h + 1) * 128,
                             c * 128:(c + 1) * 128, t]  # [co, ci]
                    wt[:, _bidx(ki, h, t, c) * 128:
                       (_bidx(ki, h, t, c) + 1) * 128] = blk.T
    wt = wt.astype(BF16)

    # --- fc weights in device feature order; fc_b via constant-1 feature ---
    # f' in [0, 2304): ch = ki*6+seg*2+h, p = co_local
    #   orig col = 600 + ki*768 + (h*128+p)*3 + seg
    # f' in [2304, 2904): orig col = f' - 2304   (e1, e2)
    # f' == 2904: constant-1 -> fc_b
    fcw = np.zeros((NCHK * 128, VR), np.float32)
    fp = np.arange(F_CONV)
    ch = fp // 128
    p = fp % 128
    ki = ch // 6
    seg = (ch % 6) // 2
    h = ch % 2
    orig = 600 + ki * 768 + (h * 128 + p) * 3 + seg
    fcw[fp] = fc_w[:, orig].T
    fcw[F_CONV:F_CONV + 600] = fc_w[:, :600].T
    fcw[F_CONV + 600] = fc_b
    fcw_host = np.ascontiguousarray(
        fcw.reshape(NCHK, 128, VR).transpose(1, 0, 2)).reshape(
        128, NCHK * VR).astype(BF16)

    # --- e1/e2 + constant-1 features, fp32, per core [640, BC] ---
    e12 = np.zeros((B, 5 * 128), np.float32)
    e12[:, :300] = e1
    e12[:, 300:600] = e2
    e12[:, 600] = 1.0
    E12c = np.ascontiguousarray(
        e12.reshape(NCORE, BC, 5 * 128).transpose(0, 2, 1))

    # --- conv biases [128, 6] fp32 ---
    cb = np.zeros((128, 6), np.float32)
    for ki in range(3):
        for h in range(2):
            cb[:, ki * 2 + h] = conv_bs[ki][h * 128:(h + 1) * 128]

    in_maps = []
    for i in range(NCORE):
        in_maps.append({
            "X": Xc[i], "WM": WMc[i], "YE": YEc[i], "WT": wt,
            "FCW": fcw_host, "E12": E12c[i], "CB": cb,
        })
    return in_maps


def kernel(**inputs):
    f = {k: np.asarray(v) for k, v in inputs.items()}
    in_maps = _prep_inputs(
        f["W"].astype(np.float32), f["e1"].astype(np.float32),
        f["e2"].astype(np.float32), f["pos_emb1"].astype(np.float32),
        f["pos_emb2"].astype(np.float32),
        [f["conv_w3"], f["conv_w5"], f["conv_w7"]],
        [f["conv_b3"], f["conv_b5"], f["conv_b7"]],
        f["fc_w"].astype(np.float32), f["fc_b"].astype(np.float32),
        f["W_pos1"], f["W_pos2"], f["e1_p"], f["e2_p"])

    from concourse.bass_utils import run_bass_kernel_spmd
    nc = _get_program()
    try:
        res = run_bass_kernel_spmd(nc, in_maps, core_ids=list(range(NCORE)))
    except Exception:
        # device wedge (e.g. NRT_EXEC_UNIT_UNRECOVERABLE left by a prior
        # crashed process): reset the runtime, then retry once
        try:
            import ctypes
            import jax
            jax.devices()
            ctypes.CDLL("/opt/axon/libaxon_pjrt.so").axon_reset()
        except Exception:
            pass
        res = run_bass_kernel_spmd(nc, in_maps, core_ids=list(range(NCORE)))
    out = np.concatenate([res.results[i]["OUT"] for i in range(NCORE)],
                         axis=0)
    return out.astype(np.float32)



# revision 18
# speedup vs baseline: 1.0129x; 1.0129x over previous
"""Trainium2 Bass kernel for the CNN/segment-reduce model.

Strategy (pure data-parallel over batch, 8 cores x 64 batch elems):
  host:   the conv over the 100 pos-embedding channels is an exact table
          lookup (conv_w @ emb gathered by index), computed on host along
          with the 4 leftover W channels (ci 896..999) -> ye term; the
          device contraction shrinks from 8 to 7 ci-chunks (-12.5% PE).
          Remaining host prep: transpose/zero-pad conv input [896, 134],
          segment weight-masks, fc_w column reorder (fc_b folded via a
          constant-1 feature), PE-facing data in bf16.
  device: conv = PE matmuls, contraction over 7 ci-chunks x k taps with
          the [128ci,128co] weight block stationary; rhs = shifted x
          window over 4 batch elems (N=508); accumulate in PSUM.
          DVE adds the host ye tile to PSUM (th_in f32), ACT tanh(+bias)
          -> bf16; DVE mask-multiply + segment-reduce -> feature tile;
          FC = 23 accumulating matmuls into PSUM [64b, 19], issued
          fine-grained per conv group for the last batch half to shrink
          the PE tail; fused exp/sum softmax; DMA out fp32.
"""

import numpy as np
import ml_dtypes

B, S, DW, DP, DC, VP, VR = 512, 128, 300, 50, 256, 256, 19
KS = (3, 5, 7)
CIN = 3 * DW + 2 * DP  # 1000
CDEV = 896             # ci channels contracted on device (7 chunks)
NCH = CDEV // 128      # 7 contraction chunks
NCORE = 8
BC = B // NCORE        # 64 batch elems per core
NB = 16                # batch elems per resident x group
NBG = BC // NB         # 4 groups
NQ = NB // 4           # 4 psum quads (4 b per N=508 matmul)
LW = S + 6             # 3 zero cols each side
PAD = 3
SL = S - 1             # conv cols actually consumed (l=127 never pooled)
NF = 4 * SL            # matmul free size (4 batch elems)
NSETS = sum(k * NCH for k in KS) * 2          # 210 weight blocks
NCHK = 23                                     # feature chunks of 128
F_CONV = 18 * 128                             # 2304 conv features
BF16 = ml_dtypes.bfloat16

# weight block index: ordered (ki-major, h, t, c)
_PREFIX = {}
_off = 0
for _ki, _k in enumerate(KS):
    for _h in range(2):
        _PREFIX[(_ki, _h)] = _off
        _off += _k * NCH


def _bidx(ki, h, t, c):
    # c-major within a group: matches the device's set consumption order so
    # a partial weight DMA unblocks the first matmuls immediately
    return _PREFIX[(ki, h)] + c * KS[ki] + t


def _build_program():
    from contextlib import ExitStack
    import concourse.tile as tile
    from concourse import bacc, mybir

    f32 = mybir.dt.float32
    bf16 = mybir.dt.bfloat16
    AF = mybir.ActivationFunctionType
    ALU = mybir.AluOpType
    AX = mybir.AxisListType

    nc = bacc.Bacc("TRN2", target_bir_lowering=False, debug=False,
                   num_devices=NCORE)

    X = nc.declare_dram_parameter("X", [NBG, NCH, 128, NB * LW], bf16,
                                  isOutput=False)
    WM = nc.declare_dram_parameter("WM", [NBG, 128, 3 * NB * SL], bf16,
                                   isOutput=False)
    YE = nc.declare_dram_parameter("YE", [NBG, 6, 128, NB * SL], bf16,
                                   isOutput=False)
    WT = nc.declare_dram_parameter("WT", [128, NSETS * 128], bf16,
                                   isOutput=False)
    FCW = nc.declare_dram_parameter("FCW", [128, NCHK * VR], bf16,
                                    isOutput=False)
    E12 = nc.declare_dram_parameter("E12", [5 * 128, BC], f32, isOutput=False)
    CB = nc.declare_dram_parameter("CB", [128, 6], f32, isOutput=False)
    OUT = nc.declare_dram_parameter("OUT", [BC, VR], f32, isOutput=True)

    with tile.TileContext(nc) as tc, ExitStack() as ctx:
        const = ctx.enter_context(tc.tile_pool(name="const", bufs=1))
        xpool = ctx.enter_context(tc.tile_pool(name="xp", bufs=2))
        mpool = ctx.enter_context(tc.tile_pool(name="mp", bufs=2))
        yepool = ctx.enter_context(tc.tile_pool(name="yep", bufs=4))
        tpool = ctx.enter_context(tc.tile_pool(name="tp", bufs=4))
        thpool = ctx.enter_context(tc.tile_pool(name="thp", bufs=4))
        prpool = ctx.enter_context(tc.tile_pool(name="prp", bufs=4))
        pspool = ctx.enter_context(tc.tile_pool(name="psp", bufs=8,
                                                space="PSUM"))

        # PE warm-up: dummy matmuls with no DMA deps so the HAM clock-gate
        # flips to 8/8 while the first weight/x DMAs are in flight.
        warm = const.tile([128, NF], bf16)
        nc.any.memset(warm[:], 0.0)
        wps = pspool.tile([128, NF], f32, tag="ps")
        for _ in range(12):
            nc.tensor.matmul(wps[:], warm[:, 0:128], warm[:],
                             start=True, stop=True)

        wt_sb = const.tile([128, NSETS * 128], bf16)
        wt_bounds = sorted(_PREFIX.values()) + [NSETS]
        fcw_sb = const.tile([128, NCHK * VR], bf16)
        cb_sb = const.tile([128, 6], f32)
        feat32 = const.tile([128, NCHK * BC], f32)
        featbf = const.tile([128, NCHK * BC], bf16)

        fcps = pspool.tile([BC, VR], f32, tag="ps")
        mx = const.tile([BC, 1], f32)
        esm = const.tile([BC, VR], f32)
        ssum = const.tile([BC, 1], f32)
        rin = const.tile([BC, 1], f32)
        osb = const.tile([BC, VR], f32)

        def softmax_rows(b0, b1):
            # softmax + output DMA for batch rows [b0, b1); row half 0 runs
            # mid-kernel as soon as its FC accumulation closes.  No max
            # subtraction: |logits| < ~5 here, exp is safe in f32, and
            # skipping it drops a reduce + the ACT accumulator drain from
            # the end-of-kernel critical chain.
            nc.scalar.activation(esm[b0:b1, :], fcps[b0:b1, :], AF.Exp)
            nc.vector.tensor_reduce(ssum[b0:b1], esm[b0:b1, :], axis=AX.X,
                                    op=ALU.add)
            nc.vector.reciprocal(rin[b0:b1], ssum[b0:b1])
            nc.vector.tensor_scalar_mul(osb[b0:b1, :], esm[b0:b1, :],
                                        rin[b0:b1])
            nc.sync.dma_start(OUT.ap()[b0:b1, :], osb[b0:b1, :])

        for bg in range(NBG):
            x_sb = xpool.tile([128, NCH * NB * LW], bf16, tag="x",
                              name=f"x_sb_{bg}")
            if bg == 0:
                # x chunk 0 (first quad first) + the first few weight
                # blocks race in; blocks are in consumption order, so 6
                # blocks cover the first two x chunks' worth of matmuls
                nc.sync.dma_start(x_sb[:, 0:4 * LW], X.ap()[0, 0][:, 0:4 * LW])
                nc.sync.dma_start(wt_sb[:, 0:6 * 128], WT.ap()[:, 0:6 * 128])
                nc.sync.dma_start(x_sb[:, 4 * LW:NB * LW],
                                  X.ap()[0, 0][:, 4 * LW:NB * LW])
                # x chunk 1 ahead of the bulk weight blocks: the c-major
                # matmul order consumes x chunks faster than wt groups
                nc.sync.dma_start(x_sb[:, NB * LW:2 * NB * LW],
                                  X.ap()[0, 1])
                nc.sync.dma_start(cb_sb[:], CB.ap()[:])
                nc.sync.dma_start(wt_sb[:, 6 * 128:wt_bounds[1] * 128],
                                  WT.ap()[:, 6 * 128:wt_bounds[1] * 128])
            for c in range(0 if bg else 2, NCH):
                nc.sync.dma_start(x_sb[:, c * NB * LW:(c + 1) * NB * LW],
                                  X.ap()[bg, c])
            wm_sb = mpool.tile([128, 3 * NB * SL], bf16, tag="wm",
                               name=f"wm_sb_{bg}")
            nc.sync.dma_start(wm_sb[:], WM.ap()[bg])
            ye_g0 = yepool.tile([128, NB * SL], bf16, tag="ye",
                                name=f"ye_{bg}_0")
            nc.sync.dma_start(ye_g0[:], YE.ap()[bg, 0])
            if bg == 0:
                # stream the rest of the weights in behind x/wm for bg0
                for wi in range(1, len(wt_bounds) - 1):
                    nc.sync.dma_start(
                        wt_sb[:, wt_bounds[wi] * 128:wt_bounds[wi + 1] * 128],
                        WT.ap()[:, wt_bounds[wi] * 128:wt_bounds[wi + 1] * 128])
                nc.sync.dma_start(fcw_sb[:], FCW.ap()[:])
                for j in range(5):
                    nc.sync.dma_start(
                        feat32[:, (18 + j) * BC:(19 + j) * BC],
                        E12.ap()[j * 128:(j + 1) * 128, :])
            xv = x_sb[:].rearrange("p (c b w) -> p c b w", c=NCH, b=NB)

            def pool_span(bg, ki, h, b0, nb, ps, ye_sb):
                # mask-pool nb batch elems starting at b0 (within this bg)
                g = ki * 2 + h
                nf = nb * SL
                th_in = thpool.tile([128, nf], f32, tag="ti",
                                    name=f"ti_{bg}_{ki}_{h}_{b0}")
                nc.vector.tensor_tensor(th_in[:], ps[:],
                                        ye_sb[:, b0 * SL:(b0 + nb) * SL],
                                        op=ALU.add)
                th = tpool.tile([128, nf], bf16, tag="th",
                                name=f"th_{bg}_{ki}_{h}_{b0}")
                nc.scalar.activation(th[:], th_in[:], AF.Tanh,
                                     bias=cb_sb[:, g:g + 1])
                for seg in range(3):
                    ch = ki * 6 + seg * 2 + h
                    col0 = ch * BC + bg * NB + b0
                    wmq = wm_sb[:, seg * NB * SL + b0 * SL:
                                seg * NB * SL + (b0 + nb) * SL]
                    pr = prpool.tile([128, nf], bf16, tag="pr",
                                     name=f"pr_{bg}_{ki}_{h}_{b0}_{seg}")
                    nc.vector.tensor_tensor(pr[:], th[:], wmq, op=ALU.mult)
                    nc.vector.tensor_reduce(
                        feat32[:, col0:col0 + nb],
                        pr[:].rearrange("p (b w) -> p b w", w=SL),
                        axis=AX.X, op=ALU.add)

            def conv_group(bg, ki, h, ye_sb, fine_tail=False):
                k = KS[ki]
                nsets = k * NCH
                # c-major: early sets touch only early x chunks, so the
                # cold-start conv isn't gated on the full x DMA
                sets = [(t, c) for c in range(NCH) for t in range(k)]

                def mm(ps, b0, nb, si, t, c):
                    s = t - k // 2
                    bi = _bidx(ki, h, t, c)
                    rhs = xv[:, c, b0:b0 + nb, PAD + s:PAD + s + SL]
                    nc.tensor.matmul(ps[:], wt_sb[:, bi * 128:(bi + 1) * 128],
                                     rhs, start=(si == 0),
                                     stop=(si == nsets - 1))

                if fine_tail:
                    # final group: span-outer staggered with shrinking span
                    # widths, so the pooling chain exposed after the very
                    # last matmul is as short as possible
                    spans = [(0, 4), (4, 4), (8, 4), (12, 2), (14, 1),
                             (15, 1)]
                    for b0, nb in spans:
                        ps = pspool.tile([128, nb * SL], f32, tag="ps",
                                         name=f"psd_{bg}_{ki}_{h}_{b0}")
                        for si, (t, c) in enumerate(sets):
                            mm(ps, b0, nb, si, t, c)
                        pool_span(bg, ki, h, b0, nb, ps, ye_sb)
                else:
                    pss = [pspool.tile([128, NF], f32, tag="ps",
                                       name=f"ps_{bg}_{ki}_{h}_{q}")
                           for q in range(NQ)]
                    for si, (t, c) in enumerate(sets):
                        for q in range(NQ):
                            mm(pss[q], q * 4, 4, si, t, c)
                    for q in range(NQ):
                        pool_span(bg, ki, h, q * 4, 4, pss[q], ye_sb)

            def cast_cols(b0, nb):
                nc.vector.tensor_copy(
                    featbf[:].rearrange("p (c b) -> p c b",
                                        c=NCHK)[:, :, b0:b0 + nb],
                    feat32[:].rearrange("p (c b) -> p c b",
                                        c=NCHK)[:, :, b0:b0 + nb])

            def cast_grp(ki, h, b0, nb):
                # cast only this conv group's 3 feature chunks for b0..b0+nb
                ch0 = ki * 6 + h
                nc.vector.tensor_copy(
                    featbf[:].rearrange("p (c b) -> p c b",
                                        c=NCHK)[:, ch0:ch0 + 5:2, b0:b0 + nb],
                    feat32[:].rearrange("p (c b) -> p c b",
                                        c=NCHK)[:, ch0:ch0 + 5:2, b0:b0 + nb])

            def cast_e12(b0, nb):
                nc.vector.tensor_copy(
                    featbf[:].rearrange("p (c b) -> p c b",
                                        c=NCHK)[:, 18:, b0:b0 + nb],
                    feat32[:].rearrange("p (c b) -> p c b",
                                        c=NCHK)[:, 18:, b0:b0 + nb])

            def fc_mm(rows, ch, start, stop):
                b0, b1 = rows
                nc.tensor.matmul(
                    fcps[b0:b1, :],
                    featbf[:, ch * BC + b0:ch * BC + b1],
                    fcw_sb[:, ch * VR:(ch + 1) * VR],
                    start=start, stop=stop)

            def fc_half(half):
                b0 = half * 32
                for ch in range(NCHK):
                    fc_mm((b0, b0 + 32), ch, ch == 0, ch == NCHK - 1)

            for gi, (ki, h) in enumerate([(ki, h) for ki in range(3)
                                          for h in range(2)]):
                final = (bg == NBG - 1) and (ki, h) == (2, 1)
                if gi > 0:
                    ye_sb = yepool.tile([128, NB * SL], bf16, tag="ye",
                                        name=f"ye_{bg}_{gi}")
                    nc.sync.dma_start(ye_sb[:], YE.ap()[bg, gi])
                else:
                    ye_sb = ye_g0
                conv_group(bg, ki, h, ye_sb, fine_tail=final)
                if bg == 2 and gi == 0:
                    # batch half 0 fully pooled at end of bg1; cast dep is
                    # ready by now so no PE bubble
                    cast_cols(0, 32)
                    fc_half(0)
                if bg == 2 and gi == 1:
                    softmax_rows(0, 32)
                if bg == 3:
                    # fine-grained FC for b32..63: e1/e2 chunks up front,
                    # then each conv group's 3 chunks as soon as the group
                    # is pooled -> tiny PE tail after the last conv matmul
                    if gi == 0:
                        cast_cols(32, 16)  # b 32..47 pooled at end of bg2
                        cast_e12(48, 16)
                        for j, ch in enumerate(range(18, 23)):
                            fc_mm((32, 64), ch, j == 0, False)
                    cast_grp(ki, h, 48, 16)
                    for j, seg in enumerate(range(3)):
                        ch = ki * 6 + seg * 2 + h
                        fc_mm((32, 64), ch, False,
                              gi == 5 and j == 2)

        softmax_rows(32, BC)

    nc.compile()
    return nc


_NC_CACHE = []


def _get_program():
    if not _NC_CACHE:
        _NC_CACHE.append(_build_program())
    return _NC_CACHE[0]


def _host_ye(W, pos_emb1, pos_emb2, conv_ws, W_pos1, W_pos2):
    """Exact host-side conv contribution of ci 896..999: the 100
    pos-embedding channels collapse to per-tap table lookups
    (conv_w @ emb.T gathered by index), plus a 4-channel einsum."""
    idx1 = W_pos1.astype(np.int64)
    idx2 = W_pos2.astype(np.int64)
    e1m = pos_emb1.astype(np.float32)
    e2m = pos_emb2.astype(np.float32)
    W4 = W[:, :, 896:900].astype(np.float32)       # [B, S, 4]
    ye = np.empty((B, 6, 128, SL), np.float32)
    for ki, k in enumerate(KS):
        cw = conv_ws[ki].astype(np.float32)        # [DC, CIN, k]
        y = np.zeros((B, SL, DC), np.float32)
        for t in range(k):
            s = t - k // 2
            l0, l1 = max(0, -s), min(SL, S - s)
            T1 = np.ascontiguousarray((cw[:, 900:950, t] @ e1m.T).T)
            T2 = np.ascontiguousarray((cw[:, 950:1000, t] @ e2m.T).T)
            y[:, l0:l1] += T1[idx1[:, l0 + s:l1 + s]]
            y[:, l0:l1] += T2[idx2[:, l0 + s:l1 + s]]
            y[:, l0:l1] += W4[:, l0 + s:l1 + s] @ cw[:, 896:900, t].T
        for h in range(2):
            ye[:, ki * 2 + h] = y[:, :, h * 128:(h + 1) * 128].transpose(
                0, 2, 1)
    return ye


def _prep_inputs(W, e1, e2, pos_emb1, pos_emb2, conv_ws, conv_bs, fc_w, fc_b,
                 W_pos1, W_pos2, e1_p, e2_p):
    """Host-side data layout; returns per-core input maps."""
    # --- conv input: [B, 896, 134] zero-padded, 3 zero cols each side ---
    Xf = W[:, :, :CDEV].transpose(0, 2, 1)         # [B, 896, S]
    Xpad = np.zeros((B, CDEV, LW), np.float32)
    Xpad[:, :, PAD:PAD + S] = Xf
    Xpad = Xpad.astype(BF16).reshape(NCORE, NBG, NB, NCH, 128, LW)
    Xc = np.ascontiguousarray(Xpad.transpose(0, 1, 3, 4, 2, 5)).reshape(
        NCORE, NBG, NCH, 128, NB * LW)

    # --- host ye: pos-emb table-lookup conv + leftover 4 channels ---
    ye = _host_ye(W, pos_emb1, pos_emb2, conv_ws, W_pos1, W_pos2)
    ye = ye.astype(BF16).reshape(NCORE, NBG, NB, 6, 128, SL)
    YEc = np.ascontiguousarray(ye.transpose(0, 1, 3, 4, 2, 5)).reshape(
        NCORE, NBG, 6, 128, NB * SL)

    # --- segment weight masks m/cnt, replicated over 128 partitions ---
    d1 = np.minimum(e1_p, e2_p).astype(np.int64)
    d2 = np.maximum(e1_p, e2_p).astype(np.int64)
    idx = np.arange(S)[None, :]
    m1 = (idx < d1[:, None])
    m2 = (idx >= d1[:, None]) & (idx < d2[:, None])
    m3 = (idx >= d2[:, None]) & (idx < S - 1)
    wm = np.stack([m1, m2, m3], axis=1).astype(np.float32)  # [B,3,S]
    cnt = np.maximum(wm.sum(axis=2), 1.0)
    wm /= cnt[:, :, None]
    wm = wm[:, :, :SL]  # l=127 is never pooled
    wm = wm.astype(BF16).reshape(NCORE, NBG, NB, 3, SL)
    wm = np.ascontiguousarray(wm.transpose(0, 1, 3, 2, 4)).reshape(
        NCORE, NBG, 1, 3 * NB * SL)
    WMc = np.ascontiguousarray(np.broadcast_to(
        wm, (NCORE, NBG, 128, 3 * NB * SL)))

    # --- conv weights -> stationary blocks [128ci, 128co], bf16 ---
    wt = np.zeros((128, NSETS * 128), np.float32)
    for ki, k in enumerate(KS):
        cw = conv_ws[ki][:, :CDEV, :]
        for h in range(2):
            for t in range(k):
                for c in range(NCH):
                    blk = cw[h * 128:(h + 1) * 128,
                             c * 128:(c + 1) * 128, t]  # [co, ci]
                    wt[:, _bidx(ki, h, t, c) * 128:
                       (_bidx(ki, h, t, c) + 1) * 128] = blk.T
    wt = wt.astype(BF16)

    # --- fc weights in device feature order; fc_b via constant-1 feature ---
    # f' in [0, 2304): ch = ki*6+seg*2+h, p = co_local
    #   orig col = 600 + ki*768 + (h*128+p)*3 + seg
    # f' in [2304, 2904): orig col = f' - 2304   (e1, e2)
    # f' == 2904: constant-1 -> fc_b
    fcw = np.zeros((NCHK * 128, VR), np.float32)
    fp = np.arange(F_CONV)
    ch = fp // 128
    p = fp % 128
    ki = ch // 6
    seg = (ch % 6) // 2
    h = ch % 2
    orig = 600 + ki * 768 + (h * 128 + p) * 3 + seg
    fcw[fp] = fc_w[:, orig].T
    fcw[F_CONV:F_CONV + 600] = fc_w[:, :600].T
    fcw[F_CONV + 600] = fc_b
    fcw_host = np.ascontiguousarray(
        fcw.reshape(NCHK, 128, VR).transpose(1, 0, 2)).reshape(
        128, NCHK * VR).astype(BF16)

    # --- e1/e2 + constant-1 features, fp32, per core [640, BC] ---
    e12 = np.zeros((B, 5 * 128), np.float32)
    e12[:, :300] = e1
    e12[:, 300:600] = e2
    e12[:, 600] = 1.0
    E12c = np.ascontiguousarray(
        e12.reshape(NCORE, BC, 5 * 128).transpose(0, 2, 1))

    # --- conv biases [128, 6] fp32 ---
    cb = np.zeros((128, 6), np.float32)
    for ki in range(3):
        for h in range(2):
            cb[:, ki * 2 + h] = conv_bs[ki][h * 128:(h + 1) * 128]

    in_maps = []
    for i in range(NCORE):
        in_maps.append({
            "X": Xc[i], "WM": WMc[i], "YE": YEc[i], "WT": wt,
            "FCW": fcw_host, "E12": E12c[i], "CB": cb,
        })
    return in_maps


def kernel(**inputs):
    f = {k: np.asarray(v) for k, v in inputs.items()}
    in_maps = _prep_inputs(
        f["W"].astype(np.float32), f["e1"].astype(np.float32),
        f["e2"].astype(np.float32), f["pos_emb1"].astype(np.float32),
        f["pos_emb2"].astype(np.float32),
        [f["conv_w3"], f["conv_w5"], f["conv_w7"]],
        [f["conv_b3"], f["conv_b5"], f["conv_b7"]],
        f["fc_w"].astype(np.float32), f["fc_b"].astype(np.float32),
        f["W_pos1"], f["W_pos2"], f["e1_p"], f["e2_p"])

    from concourse.bass_utils import run_bass_kernel_spmd
    nc = _get_program()
    try:
        res = run_bass_kernel_spmd(nc, in_maps, core_ids=list(range(NCORE)))
    except Exception:
        # device wedge (e.g. NRT_EXEC_UNIT_UNRECOVERABLE left by a prior
        # crashed process): reset the runtime, then retry once
        try:
            import ctypes
            import jax
            jax.devices()
            ctypes.CDLL("/opt/axon/libaxon_pjrt.so").axon_reset()
        except Exception:
            pass
        res = run_bass_kernel_spmd(nc, in_maps, core_ids=list(range(NCORE)))
    out = np.concatenate([res.results[i]["OUT"] for i in range(NCORE)],
                         axis=0)
    return out.astype(np.float32)



# revision 20
# speedup vs baseline: 1.0135x; 1.0006x over previous
"""Trainium2 Bass kernel for the CNN/segment-reduce model.

Strategy (pure data-parallel over batch, 8 cores x 64 batch elems):
  host:   the conv over the 100 pos-embedding channels is an exact table
          lookup (conv_w @ emb gathered by index), computed on host along
          with the 4 leftover W channels (ci 896..999) -> ye term; the
          device contraction shrinks from 8 to 7 ci-chunks (-12.5% PE).
          Remaining host prep: transpose/zero-pad conv input [896, 134],
          segment weight-masks, fc_w column reorder (fc_b folded via a
          constant-1 feature), PE-facing data in bf16.
  device: conv = PE matmuls, contraction over 7 ci-chunks x k taps with
          the [128ci,128co] weight block stationary; rhs = shifted x
          window over 4 batch elems (N=508); accumulate in PSUM.
          DVE adds the host ye tile to PSUM (th_in f32), ACT tanh(+bias)
          -> bf16; DVE mask-multiply + segment-reduce -> feature tile;
          FC = 23 accumulating matmuls into PSUM [64b, 19], issued
          fine-grained per conv group for the last batch half to shrink
          the PE tail; fused exp/sum softmax; DMA out fp32.
"""

import numpy as np
import ml_dtypes

B, S, DW, DP, DC, VP, VR = 512, 128, 300, 50, 256, 256, 19
KS = (3, 5, 7)
CIN = 3 * DW + 2 * DP  # 1000
CDEV = 896             # ci channels contracted on device (7 chunks)
NCH = CDEV // 128      # 7 contraction chunks
NCORE = 8
BC = B // NCORE        # 64 batch elems per core
NB = 16                # batch elems per resident x group
NBG = BC // NB         # 4 groups
NQ = NB // 4           # 4 psum quads (4 b per N=508 matmul)
LW = S + 6             # 3 zero cols each side
PAD = 3
SL = S - 1             # conv cols actually consumed (l=127 never pooled)
NF = 4 * SL            # matmul free size (4 batch elems)
NSETS = sum(k * NCH for k in KS) * 2          # 210 weight blocks
NCHK = 23                                     # feature chunks of 128
F_CONV = 18 * 128                             # 2304 conv features
BF16 = ml_dtypes.bfloat16

# weight block index: ordered (ki-major, h, t, c)
_PREFIX = {}
_off = 0
for _ki, _k in enumerate(KS):
    for _h in range(2):
        _PREFIX[(_ki, _h)] = _off
        _off += _k * NCH


def _bidx(ki, h, t, c):
    # c-major within a group: matches the device's set consumption order so
    # a partial weight DMA unblocks the first matmuls immediately
    return _PREFIX[(ki, h)] + c * KS[ki] + t


def _build_program():
    from contextlib import ExitStack
    import concourse.tile as tile
    from concourse import bacc, mybir

    f32 = mybir.dt.float32
    bf16 = mybir.dt.bfloat16
    AF = mybir.ActivationFunctionType
    ALU = mybir.AluOpType
    AX = mybir.AxisListType

    nc = bacc.Bacc("TRN2", target_bir_lowering=False, debug=False,
                   num_devices=NCORE)

    X = nc.declare_dram_parameter("X", [NBG, NCH, 128, NB * LW], bf16,
                                  isOutput=False)
    WM = nc.declare_dram_parameter("WM", [NBG, 128, 3 * NB * SL], bf16,
                                   isOutput=False)
    YE = nc.declare_dram_parameter("YE", [NBG, 6, 128, NB * SL], bf16,
                                   isOutput=False)
    WT = nc.declare_dram_parameter("WT", [128, NSETS * 128], bf16,
                                   isOutput=False)
    FCW = nc.declare_dram_parameter("FCW", [128, NCHK * VR], bf16,
                                    isOutput=False)
    E12 = nc.declare_dram_parameter("E12", [5 * 128, BC], f32, isOutput=False)
    CB = nc.declare_dram_parameter("CB", [128, 6], f32, isOutput=False)
    OUT = nc.declare_dram_parameter("OUT", [BC, VR], f32, isOutput=True)

    with tile.TileContext(nc) as tc, ExitStack() as ctx:
        const = ctx.enter_context(tc.tile_pool(name="const", bufs=1))
        xpool = ctx.enter_context(tc.tile_pool(name="xp", bufs=2))
        mpool = ctx.enter_context(tc.tile_pool(name="mp", bufs=2))
        yepool = ctx.enter_context(tc.tile_pool(name="yep", bufs=4))
        tpool = ctx.enter_context(tc.tile_pool(name="tp", bufs=4))
        thpool = ctx.enter_context(tc.tile_pool(name="thp", bufs=4))
        prpool = ctx.enter_context(tc.tile_pool(name="prp", bufs=4))
        pspool = ctx.enter_context(tc.tile_pool(name="psp", bufs=8,
                                                space="PSUM"))

        # PE warm-up: dummy matmuls with no DMA deps so the HAM clock-gate
        # flips to 8/8 while the first weight/x DMAs are in flight.
        warm = const.tile([128, NF], bf16)
        nc.any.memset(warm[:], 0.0)
        wps = pspool.tile([128, NF], f32, tag="ps")
        for _ in range(12):
            # N=128 keeps the HAM ramp ticking while costing ~55ns each;
            # the idle until the first DMA-fed matmul stays well under the
            # ~3.4us re-throttle window
            nc.tensor.matmul(wps[:, 0:128], warm[:, 0:128], warm[:, 0:128],
                             start=True, stop=True)

        wt_sb = const.tile([128, NSETS * 128], bf16)
        wt_bounds = sorted(_PREFIX.values()) + [NSETS]
        fcw_sb = const.tile([128, NCHK * VR], bf16)
        cb_sb = const.tile([128, 6], f32)
        feat32 = const.tile([128, NCHK * BC], f32)
        featbf = const.tile([128, NCHK * BC], bf16)

        fcps = pspool.tile([BC, VR], f32, tag="ps")
        mx = const.tile([BC, 1], f32)
        esm = const.tile([BC, VR], f32)
        ssum = const.tile([BC, 1], f32)
        rin = const.tile([BC, 1], f32)
        osb = const.tile([BC, VR], f32)

        def softmax_rows(b0, b1):
            # softmax + output DMA for batch rows [b0, b1); row half 0 runs
            # mid-kernel as soon as its FC accumulation closes.  No max
            # subtraction: |logits| < ~5 here, exp is safe in f32, and
            # skipping it drops a reduce + the ACT accumulator drain from
            # the end-of-kernel critical chain.
            nc.scalar.activation(esm[b0:b1, :], fcps[b0:b1, :], AF.Exp)
            nc.vector.tensor_reduce(ssum[b0:b1], esm[b0:b1, :], axis=AX.X,
                                    op=ALU.add)
            nc.vector.reciprocal(rin[b0:b1], ssum[b0:b1])
            nc.vector.tensor_scalar_mul(osb[b0:b1, :], esm[b0:b1, :],
                                        rin[b0:b1])
            nc.sync.dma_start(OUT.ap()[b0:b1, :], osb[b0:b1, :])

        for bg in range(NBG):
            x_sb = xpool.tile([128, NCH * NB * LW], bf16, tag="x",
                              name=f"x_sb_{bg}")
            if bg == 0:
                # x chunk 0 (first quad first) + the first few weight
                # blocks race in; blocks are in consumption order, so 6
                # blocks cover the first two x chunks' worth of matmuls
                nc.sync.dma_start(x_sb[:, 0:4 * LW], X.ap()[0, 0][:, 0:4 * LW])
                nc.sync.dma_start(wt_sb[:, 0:6 * 128], WT.ap()[:, 0:6 * 128])
                nc.sync.dma_start(x_sb[:, 4 * LW:NB * LW],
                                  X.ap()[0, 0][:, 4 * LW:NB * LW])
                # x chunk 1 ahead of the bulk weight blocks: the c-major
                # matmul order consumes x chunks faster than wt groups
                nc.sync.dma_start(x_sb[:, NB * LW:2 * NB * LW],
                                  X.ap()[0, 1])
                nc.sync.dma_start(cb_sb[:], CB.ap()[:])
                nc.sync.dma_start(wt_sb[:, 6 * 128:wt_bounds[1] * 128],
                                  WT.ap()[:, 6 * 128:wt_bounds[1] * 128])
            for c in range(0 if bg else 2, NCH):
                nc.sync.dma_start(x_sb[:, c * NB * LW:(c + 1) * NB * LW],
                                  X.ap()[bg, c])
            wm_sb = mpool.tile([128, 3 * NB * SL], bf16, tag="wm",
                               name=f"wm_sb_{bg}")
            nc.sync.dma_start(wm_sb[:], WM.ap()[bg])
            ye_g0 = yepool.tile([128, NB * SL], bf16, tag="ye",
                                name=f"ye_{bg}_0")
            nc.sync.dma_start(ye_g0[:], YE.ap()[bg, 0])
            if bg == 0:
                # stream the rest of the weights in behind x/wm for bg0
                for wi in range(1, len(wt_bounds) - 1):
                    nc.sync.dma_start(
                        wt_sb[:, wt_bounds[wi] * 128:wt_bounds[wi + 1] * 128],
                        WT.ap()[:, wt_bounds[wi] * 128:wt_bounds[wi + 1] * 128])
                nc.sync.dma_start(fcw_sb[:], FCW.ap()[:])
                for j in range(5):
                    nc.sync.dma_start(
                        feat32[:, (18 + j) * BC:(19 + j) * BC],
                        E12.ap()[j * 128:(j + 1) * 128, :])
            xv = x_sb[:].rearrange("p (c b w) -> p c b w", c=NCH, b=NB)

            def pool_span(bg, ki, h, b0, nb, ps, ye_sb):
                # mask-pool nb batch elems starting at b0 (within this bg)
                g = ki * 2 + h
                nf = nb * SL
                th_in = thpool.tile([128, nf], f32, tag="ti",
                                    name=f"ti_{bg}_{ki}_{h}_{b0}")
                nc.vector.tensor_tensor(th_in[:], ps[:],
                                        ye_sb[:, b0 * SL:(b0 + nb) * SL],
                                        op=ALU.add)
                th = tpool.tile([128, nf], bf16, tag="th",
                                name=f"th_{bg}_{ki}_{h}_{b0}")
                nc.scalar.activation(th[:], th_in[:], AF.Tanh,
                                     bias=cb_sb[:, g:g + 1])
                for seg in range(3):
                    ch = ki * 6 + seg * 2 + h
                    col0 = ch * BC + bg * NB + b0
                    wmq = wm_sb[:, seg * NB * SL + b0 * SL:
                                seg * NB * SL + (b0 + nb) * SL]
                    pr = prpool.tile([128, nf], bf16, tag="pr",
                                     name=f"pr_{bg}_{ki}_{h}_{b0}_{seg}")
                    nc.vector.tensor_tensor(pr[:], th[:], wmq, op=ALU.mult)
                    nc.vector.tensor_reduce(
                        feat32[:, col0:col0 + nb],
                        pr[:].rearrange("p (b w) -> p b w", w=SL),
                        axis=AX.X, op=ALU.add)

            def conv_group(bg, ki, h, ye_sb, fine_tail=False):
                k = KS[ki]
                nsets = k * NCH
                # c-major: early sets touch only early x chunks, so the
                # cold-start conv isn't gated on the full x DMA
                sets = [(t, c) for c in range(NCH) for t in range(k)]

                def mm(ps, b0, nb, si, t, c):
                    s = t - k // 2
                    bi = _bidx(ki, h, t, c)
                    rhs = xv[:, c, b0:b0 + nb, PAD + s:PAD + s + SL]
                    nc.tensor.matmul(ps[:], wt_sb[:, bi * 128:(bi + 1) * 128],
                                     rhs, start=(si == 0),
                                     stop=(si == nsets - 1))

                if fine_tail:
                    # final group: span-outer staggered with shrinking span
                    # widths, so the pooling chain exposed after the very
                    # last matmul is as short as possible
                    spans = [(0, 4), (4, 4), (8, 4), (12, 3), (15, 1)]
                    for b0, nb in spans:
                        ps = pspool.tile([128, nb * SL], f32, tag="ps",
                                         name=f"psd_{bg}_{ki}_{h}_{b0}")
                        for si, (t, c) in enumerate(sets):
                            mm(ps, b0, nb, si, t, c)
                        pool_span(bg, ki, h, b0, nb, ps, ye_sb)
                else:
                    pss = [pspool.tile([128, NF], f32, tag="ps",
                                       name=f"ps_{bg}_{ki}_{h}_{q}")
                           for q in range(NQ)]
                    for si, (t, c) in enumerate(sets):
                        for q in range(NQ):
                            mm(pss[q], q * 4, 4, si, t, c)
                    for q in range(NQ):
                        pool_span(bg, ki, h, q * 4, 4, pss[q], ye_sb)

            def cast_cols(b0, nb):
                nc.vector.tensor_copy(
                    featbf[:].rearrange("p (c b) -> p c b",
                                        c=NCHK)[:, :, b0:b0 + nb],
                    feat32[:].rearrange("p (c b) -> p c b",
                                        c=NCHK)[:, :, b0:b0 + nb])

            def cast_grp(ki, h, b0, nb):
                # cast only this conv group's 3 feature chunks for b0..b0+nb
                ch0 = ki * 6 + h
                nc.vector.tensor_copy(
                    featbf[:].rearrange("p (c b) -> p c b",
                                        c=NCHK)[:, ch0:ch0 + 5:2, b0:b0 + nb],
                    feat32[:].rearrange("p (c b) -> p c b",
                                        c=NCHK)[:, ch0:ch0 + 5:2, b0:b0 + nb])

            def cast_e12(b0, nb):
                nc.vector.tensor_copy(
                    featbf[:].rearrange("p (c b) -> p c b",
                                        c=NCHK)[:, 18:, b0:b0 + nb],
                    feat32[:].rearrange("p (c b) -> p c b",
                                        c=NCHK)[:, 18:, b0:b0 + nb])

            def fc_mm(rows, ch, start, stop):
                b0, b1 = rows
                nc.tensor.matmul(
                    fcps[b0:b1, :],
                    featbf[:, ch * BC + b0:ch * BC + b1],
                    fcw_sb[:, ch * VR:(ch + 1) * VR],
                    start=start, stop=stop)

            def fc_half(half):
                b0 = half * 32
                for ch in range(NCHK):
                    fc_mm((b0, b0 + 32), ch, ch == 0, ch == NCHK - 1)

            for gi, (ki, h) in enumerate([(ki, h) for ki in range(3)
                                          for h in range(2)]):
                final = (bg == NBG - 1) and (ki, h) == (2, 1)
                if gi > 0:
                    ye_sb = yepool.tile([128, NB * SL], bf16, tag="ye",
                                        name=f"ye_{bg}_{gi}")
                    nc.sync.dma_start(ye_sb[:], YE.ap()[bg, gi])
                else:
                    ye_sb = ye_g0
                conv_group(bg, ki, h, ye_sb, fine_tail=final)
                if bg == 2 and gi == 0:
                    # batch half 0 fully pooled at end of bg1; cast dep is
                    # ready by now so no PE bubble
                    cast_cols(0, 32)
                    fc_half(0)
                if bg == 2 and gi == 1:
                    softmax_rows(0, 32)
                if bg == 3:
                    # fine-grained FC for b32..63: e1/e2 chunks up front,
                    # then each conv group's 3 chunks as soon as the group
                    # is pooled -> tiny PE tail after the last conv matmul
                    if gi == 0:
                        cast_cols(32, 16)  # b 32..47 pooled at end of bg2
                        cast_e12(48, 16)
                        for j, ch in enumerate(range(18, 23)):
                            fc_mm((32, 64), ch, j == 0, False)
                    cast_grp(ki, h, 48, 16)
                    for j, seg in enumerate(range(3)):
                        ch = ki * 6 + seg * 2 + h
                        fc_mm((32, 64), ch, False,
                              gi == 5 and j == 2)

        softmax_rows(32, BC)

    nc.compile()
    return nc


_NC_CACHE = []


def _get_program():
    if not _NC_CACHE:
        _NC_CACHE.append(_build_program())
    return _NC_CACHE[0]


def _host_ye(W, pos_emb1, pos_emb2, conv_ws, W_pos1, W_pos2):
    """Exact host-side conv contribution of ci 896..999: the 100
    pos-embedding channels collapse to per-tap table lookups
    (conv_w @ emb.T gathered by index), plus a 4-channel einsum."""
    idx1 = W_pos1.astype(np.int64)
    idx2 = W_pos2.astype(np.int64)
    e1m = pos_emb1.astype(np.float32)
    e2m = pos_emb2.astype(np.float32)
    W4 = W[:, :, 896:900].astype(np.float32)       # [B, S, 4]
    ye = np.empty((B, 6, 128, SL), np.float32)
    for ki, k in enumerate(KS):
        cw = conv_ws[ki].astype(np.float32)        # [DC, CIN, k]
        y = np.zeros((B, SL, DC), np.float32)
        for t in range(k):
            s = t - k // 2
            l0, l1 = max(0, -s), min(SL, S - s)
            T1 = np.ascontiguousarray((cw[:, 900:950, t] @ e1m.T).T)
            T2 = np.ascontiguousarray((cw[:, 950:1000, t] @ e2m.T).T)
            y[:, l0:l1] += T1[idx1[:, l0 + s:l1 + s]]
            y[:, l0:l1] += T2[idx2[:, l0 + s:l1 + s]]
            y[:, l0:l1] += W4[:, l0 + s:l1 + s] @ cw[:, 896:900, t].T
        for h in range(2):
            ye[:, ki * 2 + h] = y[:, :, h * 128:(h + 1) * 128].transpose(
                0, 2, 1)
    return ye


def _prep_inputs(W, e1, e2, pos_emb1, pos_emb2, conv_ws, conv_bs, fc_w, fc_b,
                 W_pos1, W_pos2, e1_p, e2_p):
    """Host-side data layout; returns per-core input maps."""
    # --- conv input: [B, 896, 134] zero-padded, 3 zero cols each side ---
    Xf = W[:, :, :CDEV].transpose(0, 2, 1)         # [B, 896, S]
    Xpad = np.zeros((B, CDEV, LW), np.float32)
    Xpad[:, :, PAD:PAD + S] = Xf
    Xpad = Xpad.astype(BF16).reshape(NCORE, NBG, NB, NCH, 128, LW)
    Xc = np.ascontiguousarray(Xpad.transpose(0, 1, 3, 4, 2, 5)).reshape(
        NCORE, NBG, NCH, 128, NB * LW)

    # --- host ye: pos-emb table-lookup conv + leftover 4 channels ---
    ye = _host_ye(W, pos_emb1, pos_emb2, conv_ws, W_pos1, W_pos2)
    ye = ye.astype(BF16).reshape(NCORE, NBG, NB, 6, 128, SL)
    YEc = np.ascontiguousarray(ye.transpose(0, 1, 3, 4, 2, 5)).reshape(
        NCORE, NBG, 6, 128, NB * SL)

    # --- segment weight masks m/cnt, replicated over 128 partitions ---
    d1 = np.minimum(e1_p, e2_p).astype(np.int64)
    d2 = np.maximum(e1_p, e2_p).astype(np.int64)
    idx = np.arange(S)[None, :]
    m1 = (idx < d1[:, None])
    m2 = (idx >= d1[:, None]) & (idx < d2[:, None])
    m3 = (idx >= d2[:, None]) & (idx < S - 1)
    wm = np.stack([m1, m2, m3], axis=1).astype(np.float32)  # [B,3,S]
    cnt = np.maximum(wm.sum(axis=2), 1.0)
    wm /= cnt[:, :, None]
    wm = wm[:, :, :SL]  # l=127 is never pooled
    wm = wm.astype(BF16).reshape(NCORE, NBG, NB, 3, SL)
    wm = np.ascontiguousarray(wm.transpose(0, 1, 3, 2, 4)).reshape(
        NCORE, NBG, 1, 3 * NB * SL)
    WMc = np.ascontiguousarray(np.broadcast_to(
        wm, (NCORE, NBG, 128, 3 * NB * SL)))

    # --- conv weights -> stationary blocks [128ci, 128co], bf16 ---
    wt = np.zeros((128, NSETS * 128), np.float32)
    for ki, k in enumerate(KS):
        cw = conv_ws[ki][:, :CDEV, :]
        for h in range(2):
            for t in range(k):
                for c in range(NCH):
                    blk = cw[h * 128:(h + 1) * 128,
                             c * 128:(c + 1) * 128, t]  # [co, ci]
                    wt[:, _bidx(ki, h, t, c) * 128:
                       (_bidx(ki, h, t, c) + 1) * 128] = blk.T
    wt = wt.astype(BF16)

    # --- fc weights in device feature order; fc_b via constant-1 feature ---
    # f' in [0, 2304): ch = ki*6+seg*2+h, p = co_local
    #   orig col = 600 + ki*768 + (h*128+p)*3 + seg
    # f' in [2304, 2904): orig col = f' - 2304   (e1, e2)
    # f' == 2904: constant-1 -> fc_b
    fcw = np.zeros((NCHK * 128, VR), np.float32)
    fp = np.arange(F_CONV)
    ch = fp // 128
    p = fp % 128
    ki = ch // 6
    seg = (ch % 6) // 2
    h = ch % 2
    orig = 600 + ki * 768 + (h * 128 + p) * 3 + seg
    fcw[fp] = fc_w[:, orig].T
    fcw[F_CONV:F_CONV + 600] = fc_w[:, :600].T
    fcw[F_CONV + 600] = fc_b
    fcw_host = np.ascontiguousarray(
        fcw.reshape(NCHK, 128, VR).transpose(1, 0, 2)).reshape(
        128, NCHK * VR).astype(BF16)

    # --- e1/e2 + constant-1 features, fp32, per core [640, BC] ---
    e12 = np.zeros((B, 5 * 128), np.float32)
    e12[:, :300] = e1
    e12[:, 300:600] = e2
    e12[:, 600] = 1.0
    E12c = np.ascontiguousarray(
        e12.reshape(NCORE, BC, 5 * 128).transpose(0, 2, 1))

    # --- conv biases [128, 6] fp32 ---
    cb = np.zeros((128, 6), np.float32)
    for ki in range(3):
        for h in range(2):
            cb[:, ki * 2 + h] = conv_bs[ki][h * 128:(h + 1) * 128]

    in_maps = []
    for i in range(NCORE):
        in_maps.append({
            "X": Xc[i], "WM": WMc[i], "YE": YEc[i], "WT": wt,
            "FCW": fcw_host, "E12": E12c[i], "CB": cb,
        })
    return in_maps


def kernel(**inputs):
    f = {k: np.asarray(v) for k, v in inputs.items()}
    in_maps = _prep_inputs(
        f["W"].astype(np.float32), f["e1"].astype(np.float32),
        f["e2"].astype(np.float32), f["pos_emb1"].astype(np.float32),
        f["pos_emb2"].astype(np.float32),
        [f["conv_w3"], f["conv_w5"], f["conv_w7"]],
        [f["conv_b3"], f["conv_b5"], f["conv_b7"]],
        f["fc_w"].astype(np.float32), f["fc_b"].astype(np.float32),
        f["W_pos1"], f["W_pos2"], f["e1_p"], f["e2_p"])

    from concourse.bass_utils import run_bass_kernel_spmd
    nc = _get_program()
    try:
        res = run_bass_kernel_spmd(nc, in_maps, core_ids=list(range(NCORE)))
    except Exception:
        # device wedge (e.g. NRT_EXEC_UNIT_UNRECOVERABLE left by a prior
        # crashed process): reset the runtime, then retry once
        try:
            import ctypes
            import jax
            jax.devices()
            ctypes.CDLL("/opt/axon/libaxon_pjrt.so").axon_reset()
        except Exception:
            pass
        res = run_bass_kernel_spmd(nc, in_maps, core_ids=list(range(NCORE)))
    out = np.concatenate([res.results[i]["OUT"] for i in range(NCORE)],
                         axis=0)
    return out.astype(np.float32)



# revision 21
# speedup vs baseline: 1.0142x; 1.0007x over previous
"""Trainium2 Bass kernel for the CNN/segment-reduce model.

Strategy (pure data-parallel over batch, 8 cores x 64 batch elems):
  host:   the conv over the 100 pos-embedding channels is an exact table
          lookup (conv_w @ emb gathered by index), computed on host along
          with the 4 leftover W channels (ci 896..999) -> ye term; the
          device contraction shrinks from 8 to 7 ci-chunks (-12.5% PE).
          Remaining host prep: transpose/zero-pad conv input [896, 134],
          segment weight-masks, fc_w column reorder (fc_b folded via a
          constant-1 feature), PE-facing data in bf16.
  device: conv = PE matmuls, contraction over 7 ci-chunks x k taps with
          the [128ci,128co] weight block stationary; rhs = shifted x
          window over 4 batch elems (N=508); accumulate in PSUM.
          DVE adds the host ye tile to PSUM (th_in f32), ACT tanh(+bias)
          -> bf16; DVE mask-multiply + segment-reduce -> feature tile;
          FC = 23 accumulating matmuls into PSUM [64b, 19], issued
          fine-grained per conv group for the last batch half to shrink
          the PE tail; fused exp/sum softmax; DMA out fp32.
"""

import numpy as np
import ml_dtypes

B, S, DW, DP, DC, VP, VR = 512, 128, 300, 50, 256, 256, 19
KS = (3, 5, 7)
CIN = 3 * DW + 2 * DP  # 1000
CDEV = 896             # ci channels contracted on device (7 chunks)
NCH = CDEV // 128      # 7 contraction chunks
NCORE = 8
BC = B // NCORE        # 64 batch elems per core
NB = 16                # batch elems per resident x group
NBG = BC // NB         # 4 groups
NQ = NB // 4           # 4 psum quads (4 b per N=508 matmul)
LW = S + 6             # 3 zero cols each side
PAD = 3
SL = S - 1             # conv cols actually consumed (l=127 never pooled)
NF = 4 * SL            # matmul free size (4 batch elems)
NSETS = sum(k * NCH for k in KS) * 2          # 210 weight blocks
NCHK = 23                                     # feature chunks of 128
F_CONV = 18 * 128                             # 2304 conv features
BF16 = ml_dtypes.bfloat16

# weight block index: ordered (ki-major, h, t, c)
_PREFIX = {}
_off = 0
for _ki, _k in enumerate(KS):
    for _h in range(2):
        _PREFIX[(_ki, _h)] = _off
        _off += _k * NCH


def _bidx(ki, h, t, c):
    # c-major within a group: matches the device's set consumption order so
    # a partial weight DMA unblocks the first matmuls immediately
    return _PREFIX[(ki, h)] + c * KS[ki] + t


def _build_program():
    from contextlib import ExitStack
    import concourse.tile as tile
    from concourse import bacc, mybir

    f32 = mybir.dt.float32
    bf16 = mybir.dt.bfloat16
    AF = mybir.ActivationFunctionType
    ALU = mybir.AluOpType
    AX = mybir.AxisListType

    nc = bacc.Bacc("TRN2", target_bir_lowering=False, debug=False,
                   num_devices=NCORE)

    X = nc.declare_dram_parameter("X", [NBG, NCH, 128, NB * LW], bf16,
                                  isOutput=False)
    WM = nc.declare_dram_parameter("WM", [NBG, 128, 3 * NB * SL], bf16,
                                   isOutput=False)
    YE = nc.declare_dram_parameter("YE", [NBG, 6, 128, NB * SL], bf16,
                                   isOutput=False)
    WT = nc.declare_dram_parameter("WT", [128, NSETS * 128], bf16,
                                   isOutput=False)
    FCW = nc.declare_dram_parameter("FCW", [128, NCHK * VR], bf16,
                                    isOutput=False)
    E12 = nc.declare_dram_parameter("E12", [5 * 128, BC], f32, isOutput=False)
    CB = nc.declare_dram_parameter("CB", [128, 6], f32, isOutput=False)
    OUT = nc.declare_dram_parameter("OUT", [BC, VR], f32, isOutput=True)

    with tile.TileContext(nc) as tc, ExitStack() as ctx:
        const = ctx.enter_context(tc.tile_pool(name="const", bufs=1))
        xpool = ctx.enter_context(tc.tile_pool(name="xp", bufs=2))
        mpool = ctx.enter_context(tc.tile_pool(name="mp", bufs=2))
        yepool = ctx.enter_context(tc.tile_pool(name="yep", bufs=4))
        tpool = ctx.enter_context(tc.tile_pool(name="tp", bufs=4))
        thpool = ctx.enter_context(tc.tile_pool(name="thp", bufs=4))
        prpool = ctx.enter_context(tc.tile_pool(name="prp", bufs=4))
        pspool = ctx.enter_context(tc.tile_pool(name="psp", bufs=8,
                                                space="PSUM"))

        # PE warm-up: dummy matmuls with no DMA deps so the HAM clock-gate
        # flips to 8/8 while the first weight/x DMAs are in flight.
        warm = const.tile([128, NF], bf16)
        nc.any.memset(warm[:], 0.0)
        wps = pspool.tile([128, NF], f32, tag="ps")
        for _ in range(12):
            # full-width: dense early PE activity ramps the HAM utilization
            # meter to 8/8 sooner than sparse short matmuls would
            nc.tensor.matmul(wps[:], warm[:, 0:128], warm[:],
                             start=True, stop=True)

        wt_sb = const.tile([128, NSETS * 128], bf16)
        wt_bounds = sorted(_PREFIX.values()) + [NSETS]
        fcw_sb = const.tile([128, NCHK * VR], bf16)
        cb_sb = const.tile([128, 6], f32)
        feat32 = const.tile([128, NCHK * BC], f32)
        featbf = const.tile([128, NCHK * BC], bf16)

        fcps = pspool.tile([BC, VR], f32, tag="ps")
        mx = const.tile([BC, 1], f32)
        esm = const.tile([BC, VR], f32)
        ssum = const.tile([BC, 1], f32)
        rin = const.tile([BC, 1], f32)
        osb = const.tile([BC, VR], f32)

        def softmax_rows(b0, b1):
            # softmax + output DMA for batch rows [b0, b1); row half 0 runs
            # mid-kernel as soon as its FC accumulation closes.  No max
            # subtraction: |logits| < ~5 here, exp is safe in f32, and
            # skipping it drops a reduce + the ACT accumulator drain from
            # the end-of-kernel critical chain.
            nc.scalar.activation(esm[b0:b1, :], fcps[b0:b1, :], AF.Exp)
            nc.vector.tensor_reduce(ssum[b0:b1], esm[b0:b1, :], axis=AX.X,
                                    op=ALU.add)
            nc.vector.reciprocal(rin[b0:b1], ssum[b0:b1])
            nc.vector.tensor_scalar_mul(osb[b0:b1, :], esm[b0:b1, :],
                                        rin[b0:b1])
            nc.sync.dma_start(OUT.ap()[b0:b1, :], osb[b0:b1, :])

        for bg in range(NBG):
            x_sb = xpool.tile([128, NCH * NB * LW], bf16, tag="x",
                              name=f"x_sb_{bg}")
            if bg == 0:
                # x chunk 0 (first quad first) + the first few weight
                # blocks race in; blocks are in consumption order, so 6
                # blocks cover the first two x chunks' worth of matmuls
                nc.sync.dma_start(x_sb[:, 0:4 * LW], X.ap()[0, 0][:, 0:4 * LW])
                nc.sync.dma_start(wt_sb[:, 0:6 * 128], WT.ap()[:, 0:6 * 128])
                nc.sync.dma_start(x_sb[:, 4 * LW:NB * LW],
                                  X.ap()[0, 0][:, 4 * LW:NB * LW])
                # x chunk 1 ahead of the bulk weight blocks: the c-major
                # matmul order consumes x chunks faster than wt groups
                nc.sync.dma_start(x_sb[:, NB * LW:2 * NB * LW],
                                  X.ap()[0, 1])
                nc.sync.dma_start(cb_sb[:], CB.ap()[:])
                nc.sync.dma_start(wt_sb[:, 6 * 128:wt_bounds[1] * 128],
                                  WT.ap()[:, 6 * 128:wt_bounds[1] * 128])
            for c in range(0 if bg else 2, NCH):
                nc.sync.dma_start(x_sb[:, c * NB * LW:(c + 1) * NB * LW],
                                  X.ap()[bg, c])
            wm_sb = mpool.tile([128, 3 * NB * SL], bf16, tag="wm",
                               name=f"wm_sb_{bg}")
            nc.sync.dma_start(wm_sb[:], WM.ap()[bg])
            ye_g0 = yepool.tile([128, NB * SL], bf16, tag="ye",
                                name=f"ye_{bg}_0")
            nc.sync.dma_start(ye_g0[:], YE.ap()[bg, 0])
            if bg == 0:
                # stream the rest of the weights in behind x/wm for bg0
                for wi in range(1, len(wt_bounds) - 1):
                    nc.sync.dma_start(
                        wt_sb[:, wt_bounds[wi] * 128:wt_bounds[wi + 1] * 128],
                        WT.ap()[:, wt_bounds[wi] * 128:wt_bounds[wi + 1] * 128])
                nc.sync.dma_start(fcw_sb[:], FCW.ap()[:])
                for j in range(5):
                    nc.sync.dma_start(
                        feat32[:, (18 + j) * BC:(19 + j) * BC],
                        E12.ap()[j * 128:(j + 1) * 128, :])
            xv = x_sb[:].rearrange("p (c b w) -> p c b w", c=NCH, b=NB)

            def pool_span(bg, ki, h, b0, nb, ps, ye_sb):
                # mask-pool nb batch elems starting at b0 (within this bg)
                g = ki * 2 + h
                nf = nb * SL
                th_in = thpool.tile([128, nf], f32, tag="ti",
                                    name=f"ti_{bg}_{ki}_{h}_{b0}")
                nc.vector.tensor_tensor(th_in[:], ps[:],
                                        ye_sb[:, b0 * SL:(b0 + nb) * SL],
                                        op=ALU.add)
                th = tpool.tile([128, nf], bf16, tag="th",
                                name=f"th_{bg}_{ki}_{h}_{b0}")
                nc.scalar.activation(th[:], th_in[:], AF.Tanh,
                                     bias=cb_sb[:, g:g + 1])
                for seg in range(3):
                    ch = ki * 6 + seg * 2 + h
                    col0 = ch * BC + bg * NB + b0
                    wmq = wm_sb[:, seg * NB * SL + b0 * SL:
                                seg * NB * SL + (b0 + nb) * SL]
                    pr = prpool.tile([128, nf], bf16, tag="pr",
                                     name=f"pr_{bg}_{ki}_{h}_{b0}_{seg}")
                    nc.vector.tensor_tensor(pr[:], th[:], wmq, op=ALU.mult)
                    nc.vector.tensor_reduce(
                        feat32[:, col0:col0 + nb],
                        pr[:].rearrange("p (b w) -> p b w", w=SL),
                        axis=AX.X, op=ALU.add)

            def conv_group(bg, ki, h, ye_sb, fine_tail=False):
                k = KS[ki]
                nsets = k * NCH
                # c-major: early sets touch only early x chunks, so the
                # cold-start conv isn't gated on the full x DMA
                sets = [(t, c) for c in range(NCH) for t in range(k)]

                def mm(ps, b0, nb, si, t, c):
                    s = t - k // 2
                    bi = _bidx(ki, h, t, c)
                    rhs = xv[:, c, b0:b0 + nb, PAD + s:PAD + s + SL]
                    nc.tensor.matmul(ps[:], wt_sb[:, bi * 128:(bi + 1) * 128],
                                     rhs, start=(si == 0),
                                     stop=(si == nsets - 1))

                if fine_tail:
                    # final group: span-outer staggered with shrinking span
                    # widths, so the pooling chain exposed after the very
                    # last matmul is as short as possible
                    spans = [(0, 4), (4, 4), (8, 4), (12, 3), (15, 1)]
                    for b0, nb in spans:
                        ps = pspool.tile([128, nb * SL], f32, tag="ps",
                                         name=f"psd_{bg}_{ki}_{h}_{b0}")
                        for si, (t, c) in enumerate(sets):
                            mm(ps, b0, nb, si, t, c)
                        pool_span(bg, ki, h, b0, nb, ps, ye_sb)
                else:
                    pss = [pspool.tile([128, NF], f32, tag="ps",
                                       name=f"ps_{bg}_{ki}_{h}_{q}")
                           for q in range(NQ)]
                    for si, (t, c) in enumerate(sets):
                        for q in range(NQ):
                            mm(pss[q], q * 4, 4, si, t, c)
                    for q in range(NQ):
                        pool_span(bg, ki, h, q * 4, 4, pss[q], ye_sb)

            def cast_cols(b0, nb):
                nc.vector.tensor_copy(
                    featbf[:].rearrange("p (c b) -> p c b",
                                        c=NCHK)[:, :, b0:b0 + nb],
                    feat32[:].rearrange("p (c b) -> p c b",
                                        c=NCHK)[:, :, b0:b0 + nb])

            def cast_grp(ki, h, b0, nb):
                # cast only this conv group's 3 feature chunks for b0..b0+nb
                ch0 = ki * 6 + h
                nc.vector.tensor_copy(
                    featbf[:].rearrange("p (c b) -> p c b",
                                        c=NCHK)[:, ch0:ch0 + 5:2, b0:b0 + nb],
                    feat32[:].rearrange("p (c b) -> p c b",
                                        c=NCHK)[:, ch0:ch0 + 5:2, b0:b0 + nb])

            def cast_e12(b0, nb):
                nc.vector.tensor_copy(
                    featbf[:].rearrange("p (c b) -> p c b",
                                        c=NCHK)[:, 18:, b0:b0 + nb],
                    feat32[:].rearrange("p (c b) -> p c b",
                                        c=NCHK)[:, 18:, b0:b0 + nb])

            def fc_mm(rows, ch, start, stop):
                b0, b1 = rows
                nc.tensor.matmul(
                    fcps[b0:b1, :],
                    featbf[:, ch * BC + b0:ch * BC + b1],
                    fcw_sb[:, ch * VR:(ch + 1) * VR],
                    start=start, stop=stop)

            def fc_half(half):
                b0 = half * 32
                for ch in range(NCHK):
                    fc_mm((b0, b0 + 32), ch, ch == 0, ch == NCHK - 1)

            for gi, (ki, h) in enumerate([(ki, h) for ki in range(3)
                                          for h in range(2)]):
                final = (bg == NBG - 1) and (ki, h) == (2, 1)
                if gi > 0:
                    ye_sb = yepool.tile([128, NB * SL], bf16, tag="ye",
                                        name=f"ye_{bg}_{gi}")
                    nc.sync.dma_start(ye_sb[:], YE.ap()[bg, gi])
                else:
                    ye_sb = ye_g0
                conv_group(bg, ki, h, ye_sb, fine_tail=final)
                if bg == 2 and gi == 0:
                    # batch half 0 fully pooled at end of bg1; cast dep is
                    # ready by now so no PE bubble
                    cast_cols(0, 32)
                    fc_half(0)
                if bg == 2 and gi == 1:
                    softmax_rows(0, 32)
                if bg == 3:
                    # fine-grained FC for b32..63: e1/e2 chunks up front,
                    # then each conv group's 3 chunks as soon as the group
                    # is pooled -> tiny PE tail after the last conv matmul
                    if gi == 0:
                        cast_cols(32, 16)  # b 32..47 pooled at end of bg2
                        cast_e12(48, 16)
                        for j, ch in enumerate(range(18, 23)):
                            fc_mm((32, 64), ch, j == 0, False)
                    cast_grp(ki, h, 48, 16)
                    for j, seg in enumerate(range(3)):
                        ch = ki * 6 + seg * 2 + h
                        fc_mm((32, 64), ch, False,
                              gi == 5 and j == 2)

        softmax_rows(32, BC)

    nc.compile()
    return nc


_NC_CACHE = []


def _get_program():
    if not _NC_CACHE:
        _NC_CACHE.append(_build_program())
    return _NC_CACHE[0]


def _host_ye(W, pos_emb1, pos_emb2, conv_ws, W_pos1, W_pos2):
    """Exact host-side conv contribution of ci 896..999: the 100
    pos-embedding channels collapse to per-tap table lookups
    (conv_w @ emb.T gathered by index), plus a 4-channel einsum."""
    idx1 = W_pos1.astype(np.int64)
    idx2 = W_pos2.astype(np.int64)
    e1m = pos_emb1.astype(np.float32)
    e2m = pos_emb2.astype(np.float32)
    W4 = W[:, :, 896:900].astype(np.float32)       # [B, S, 4]
    ye = np.empty((B, 6, 128, SL), np.float32)
    for ki, k in enumerate(KS):
        cw = conv_ws[ki].astype(np.float32)        # [DC, CIN, k]
        y = np.zeros((B, SL, DC), np.float32)
        for t in range(k):
            s = t - k // 2
            l0, l1 = max(0, -s), min(SL, S - s)
            T1 = np.ascontiguousarray((cw[:, 900:950, t] @ e1m.T).T)
            T2 = np.ascontiguousarray((cw[:, 950:1000, t] @ e2m.T).T)
            y[:, l0:l1] += T1[idx1[:, l0 + s:l1 + s]]
            y[:, l0:l1] += T2[idx2[:, l0 + s:l1 + s]]
            y[:, l0:l1] += W4[:, l0 + s:l1 + s] @ cw[:, 896:900, t].T
        for h in range(2):
            ye[:, ki * 2 + h] = y[:, :, h * 128:(h + 1) * 128].transpose(
                0, 2, 1)
    return ye


def _prep_inputs(W, e1, e2, pos_emb1, pos_emb2, conv_ws, conv_bs, fc_w, fc_b,
                 W_pos1, W_pos2, e1_p, e2_p):
    """Host-side data layout; returns per-core input maps."""
    # --- conv input: [B, 896, 134] zero-padded, 3 zero cols each side ---
    Xf = W[:, :, :CDEV].transpose(0, 2, 1)         # [B, 896, S]
    Xpad = np.zeros((B, CDEV, LW), np.float32)
    Xpad[:, :, PAD:PAD + S] = Xf
    Xpad = Xpad.astype(BF16).reshape(NCORE, NBG, NB, NCH, 128, LW)
    Xc = np.ascontiguousarray(Xpad.transpose(0, 1, 3, 4, 2, 5)).reshape(
        NCORE, NBG, NCH, 128, NB * LW)

    # --- host ye: pos-emb table-lookup conv + leftover 4 channels ---
    ye = _host_ye(W, pos_emb1, pos_emb2, conv_ws, W_pos1, W_pos2)
    ye = ye.astype(BF16).reshape(NCORE, NBG, NB, 6, 128, SL)
    YEc = np.ascontiguousarray(ye.transpose(0, 1, 3, 4, 2, 5)).reshape(
        NCORE, NBG, 6, 128, NB * SL)

    # --- segment weight masks m/cnt, replicated over 128 partitions ---
    d1 = np.minimum(e1_p, e2_p).astype(np.int64)
    d2 = np.maximum(e1_p, e2_p).astype(np.int64)
    idx = np.arange(S)[None, :]
    m1 = (idx < d1[:, None])
    m2 = (idx >= d1[:, None]) & (idx < d2[:, None])
    m3 = (idx >= d2[:, None]) & (idx < S - 1)
    wm = np.stack([m1, m2, m3], axis=1).astype(np.float32)  # [B,3,S]
    cnt = np.maximum(wm.sum(axis=2), 1.0)
    wm /= cnt[:, :, None]
    wm = wm[:, :, :SL]  # l=127 is never pooled
    wm = wm.astype(BF16).reshape(NCORE, NBG, NB, 3, SL)
    wm = np.ascontiguousarray(wm.transpose(0, 1, 3, 2, 4)).reshape(
        NCORE, NBG, 1, 3 * NB * SL)
    WMc = np.ascontiguousarray(np.broadcast_to(
        wm, (NCORE, NBG, 128, 3 * NB * SL)))

    # --- conv weights -> stationary blocks [128ci, 128co], bf16 ---
    wt = np.zeros((128, NSETS * 128), np.float32)
    for ki, k in enumerate(KS):
        cw = conv_ws[ki][:, :CDEV, :]
        for h in range(2):
            for t in range(k):
                for c in range(NCH):
                    blk = cw[h * 128:(h + 1) * 128,
                             c * 128:(c + 1) * 128, t]  # [co, ci]
                    wt[:, _bidx(ki, h, t, c) * 128:
                       (_bidx(ki, h, t, c) + 1) * 128] = blk.T
    wt = wt.astype(BF16)

    # --- fc weights in device feature order; fc_b via constant-1 feature ---
    # f' in [0, 2304): ch = ki*6+seg*2+h, p = co_local
    #   orig col = 600 + ki*768 + (h*128+p)*3 + seg
    # f' in [2304, 2904): orig col = f' - 2304   (e1, e2)
    # f' == 2904: constant-1 -> fc_b
    fcw = np.zeros((NCHK * 128, VR), np.float32)
    fp = np.arange(F_CONV)
    ch = fp // 128
    p = fp % 128
    ki = ch // 6
    seg = (ch % 6) // 2
    h = ch % 2
    orig = 600 + ki * 768 + (h * 128 + p) * 3 + seg
    fcw[fp] = fc_w[:, orig].T
    fcw[F_CONV:F_CONV + 600] = fc_w[:, :600].T
    fcw[F_CONV + 600] = fc_b
    fcw_host = np.ascontiguousarray(
        fcw.reshape(NCHK, 128, VR).transpose(1, 0, 2)).reshape(
        128, NCHK * VR).astype(BF16)

    # --- e1/e2 + constant-1 features, fp32, per core [640, BC] ---
    e12 = np.zeros((B, 5 * 128), np.float32)
    e12[:, :300] = e1
    e12[:, 300:600] = e2
    e12[:, 600] = 1.0
    E12c = np.ascontiguousarray(
        e12.reshape(NCORE, BC, 5 * 128).transpose(0, 2, 1))

    # --- conv biases [128, 6] fp32 ---
    cb = np.zeros((128, 6), np.float32)
    for ki in range(3):
        for h in range(2):
            cb[:, ki * 2 + h] = conv_bs[ki][h * 128:(h + 1) * 128]

    in_maps = []
    for i in range(NCORE):
        in_maps.append({
            "X": Xc[i], "WM": WMc[i], "YE": YEc[i], "WT": wt,
            "FCW": fcw_host, "E12": E12c[i], "CB": cb,
        })
    return in_maps


def kernel(**inputs):
    f = {k: np.asarray(v) for k, v in inputs.items()}
    in_maps = _prep_inputs(
        f["W"].astype(np.float32), f["e1"].astype(np.float32),
        f["e2"].astype(np.float32), f["pos_emb1"].astype(np.float32),
        f["pos_emb2"].astype(np.float32),
        [f["conv_w3"], f["conv_w5"], f["conv_w7"]],
        [f["conv_b3"], f["conv_b5"], f["conv_b7"]],
        f["fc_w"].astype(np.float32), f["fc_b"].astype(np.float32),
        f["W_pos1"], f["W_pos2"], f["e1_p"], f["e2_p"])

    from concourse.bass_utils import run_bass_kernel_spmd
    nc = _get_program()
    try:
        res = run_bass_kernel_spmd(nc, in_maps, core_ids=list(range(NCORE)))
    except Exception:
        # device wedge (e.g. NRT_EXEC_UNIT_UNRECOVERABLE left by a prior
        # crashed process): reset the runtime, then retry once
        try:
            import ctypes
            import jax
            jax.devices()
            ctypes.CDLL("/opt/axon/libaxon_pjrt.so").axon_reset()
        except Exception:
            pass
        res = run_bass_kernel_spmd(nc, in_maps, core_ids=list(range(NCORE)))
    out = np.concatenate([res.results[i]["OUT"] for i in range(NCORE)],
                         axis=0)
    return out.astype(np.float32)

